# revision 6
# baseline (speedup 1.0000x reference)
"""DecoderLM Trainium2 kernel (8 NeuronCores, SPMD + pairwise collectives).

Sharding: data-parallel over batch (B=4) x vocab-parallel LM head (2 halves),
with the transformer body SEQUENCE-SPLIT across each vocab pair: core c
(rank p = c//4) computes the body only for token tiles {2i+p}, i=0..3, of
batch element c%4. Per layer the pair exchanges its K/V halves with a 2-core
DRAM AllGather; both cores rebuild the gathered tensors in GLOBAL token order
from the CC buffer, so the instruction stream is identical on every core
(single SPMD program). Rank-dependent causal masking is host data:
  rank0 (even q tiles): block kt==2i -> tri,  kt==2i+1 -> zeros
  rank1 (odd  q tiles): block kt==2i -> ones, kt==2i+1 -> tri
After the last layer the pair all-gathers the final hidden state (bf16,
transposed) and each core computes logits for ALL tokens x its vocab half.

Per-core layout: activations kept as h [t, d] fp32 (residual stream, 4 own
tiles) plus a transposed copy hT [d, 512] in the matmul dtype:
  qT/kT  = W.T @ hT          (lhsT=W, rhs=hT)       [own tokens]
  v      = h @ Wv            (lhsT=hT, rhs=Wv)      [own tokens, per-head 65
                                                     cols: 64 dims + ones col]
  (kT,v) --AllGather--> kT_full/v_full in global token order
  scoresT= K @ qT (per head) (lhsT=kT_full, rhs=qT) [k-major, causal suffix]
  attnT  = exp(scoresT/8) * mask (first suffix block only)
  oT'    = v'.T @ attnT      (lhsT=v', rhs=attnT)   [65 rows: 64 out + denom]
  o_proj = oT @ Wo           (lhsT=oT, rhs=Wo)      [+= residual]
  ffT    = W1.T @ hT, gelu   (lhsT=W1, rhs=hT)
  ffn    = ffT.T @ W2        (lhsT=ffT, rhs=W2)     [+= residual]
  hTf    --AllGather--> hTf_full; logits = hTf_full.T @ Wl'  [lnf folded]
"""

import numpy as np
from contextlib import ExitStack

import concourse.bass as bass
import concourse.tile as tile
import concourse.mybir as mybir
from concourse.bass_utils import run_bass_kernel_spmd

AF = mybir.ActivationFunctionType
ALU = mybir.AluOpType
F32 = mybir.dt.float32
BF16 = mybir.dt.bfloat16

# Model dims (hardcoded per problem spec).
V, D, H, F, L, B, T = 32000, 512, 8, 2048, 4, 4, 1024
DK = D // H          # 64
NCORES = 8
VSH = V // 2         # 16000 vocab columns per core
NT = T // 128        # 8 token tiles (global)
NTO = NT // 2        # 4 own token tiles
TO = NTO * 128       # 512 own tokens
ND = D // 128        # 4 d chunks
NF = F // 128        # 16 f chunks
VG = 4               # vocab groups per core
VGW = VSH // VG      # 4000
VC = 500             # vocab columns per matmul (psum bank limit 512)
NVC = VGW // VC      # 8 chunks per group
EPS = 1e-5

VW = H * (DK + 1)    # 520 v cols per token tile
CCW = ND * TO + NTO * VW   # 2048 + 2080 = 4128 staged cols per layer
CCF = ND * TO              # 2048 staged cols for final h

GROUPS = [[0, 4], [1, 5], [2, 6], [3, 7]]

MM_DT = BF16
MM_NP = np.dtype(mybir.dt.np(MM_DT))

WAIT_CAP = 1

_counter = [0]


def _split_waits(nc, cap=WAIT_CAP):
    """This walrus build rejects instructions with >cap attached sem waits
    ("Too many sync wait commands"). Peel excess waits onto same-engine NoOps
    inserted immediately before the instruction (queue order preserved)."""
    n = 0
    for fn in nc.m.functions:
        for bb in fn.blocks:
            new_list, changed = [], False
            for ins in bb.instructions:
                si = getattr(ins, "sync_info", None)
                waits = list(si.on_wait) if (si is not None and si.on_wait) else []
                if len(waits) > cap:
                    changed = True
                    n += 1
                    excess, keep = waits[:-cap], waits[-cap:]
                    for i in range(0, len(excess), cap):
                        _counter[0] += 1
                        nop = mybir.InstNoOp(
                            name=f"I-wsplit-{_counter[0]}", ins=[], outs=[]
                        )
                        nop.engine = ins.engine
                        nop.sync_info = mybir.SyncInfo(
                            on_wait=excess[i : i + cap], on_update=[]
                        )
                        new_list.append(nop)
                    ins.sync_info = mybir.SyncInfo(
                        on_wait=keep, on_update=list(si.on_update or [])
                    )
                new_list.append(ins)
            if changed:
                bb.instructions = new_list
    return n


def build_program(flags, n_layers=L, lm=True):
    """flags: dict of booleans: bias_qkv, bias_o, bias_1, bias_2, ln1, ln2, bl
    (True = nontrivial, build the general ops)."""
    nc = bass.Bass(num_devices=NCORES)

    # ---- DRAM I/O ----
    d_idx = nc.dram_tensor("idx", [128, NTO], mybir.dt.int32, kind="ExternalInput")
    d_np = nc.dram_tensor("notpad", [128, NTO], F32, kind="ExternalInput")
    d_pe = nc.dram_tensor("pe", [NTO, 128, D], F32, kind="ExternalInput")
    d_emb = nc.dram_tensor("embed", [V, D], F32, kind="ExternalInput")
    d_mask = nc.dram_tensor("mask", [128, NTO, 2, 128], MM_DT, kind="ExternalInput")
    d_wq = nc.dram_tensor("wq", [L, 128, ND, D], MM_DT, kind="ExternalInput")
    d_wk = nc.dram_tensor("wk", [L, 128, ND, D], MM_DT, kind="ExternalInput")
    d_wv = nc.dram_tensor("wv", [L, 128, ND, D], MM_DT, kind="ExternalInput")
    d_wo = nc.dram_tensor("wo", [L, 128, ND, D], MM_DT, kind="ExternalInput")
    d_w1 = nc.dram_tensor("w1", [L, 128, ND, F], MM_DT, kind="ExternalInput")
    d_w2 = nc.dram_tensor("w2", [L, 128, NF, D], MM_DT, kind="ExternalInput")
    d_wl = nc.dram_tensor("wl", [128, ND, VG, VGW], MM_DT, kind="ExternalInput")
    d_out = nc.dram_tensor("logits", [T, VSH], F32, kind="ExternalOutput")

    # per-layer CC staging (separate tensors -> no WAR hazards across layers)
    d_cc_in = [
        nc.dram_tensor(f"ccin{l}", [128, CCW], MM_DT, kind="Internal")
        for l in range(n_layers)
    ]
    d_cc_out = [
        nc.dram_tensor(f"ccout{l}", [2, 128, CCW], MM_DT, kind="Internal")
        for l in range(n_layers)
    ]
    d_ccf_in = nc.dram_tensor("ccfin", [128, CCF], MM_DT, kind="Internal")
    d_ccf_out = nc.dram_tensor("ccfout", [2, 128, CCF], MM_DT, kind="Internal")

    ext = {}
    if flags["bias_qkv"]:
        ext["bq"] = nc.dram_tensor("bq", [L, 128, ND], F32, kind="ExternalInput")
        ext["bk"] = nc.dram_tensor("bk", [L, 128, ND], F32, kind="ExternalInput")
        ext["bvb"] = nc.dram_tensor("bvb", [L, 128, D], F32, kind="ExternalInput")
    if flags["bias_o"]:
        ext["bob"] = nc.dram_tensor("bob", [L, 128, D], F32, kind="ExternalInput")
    if flags["bias_1"]:
        ext["b1"] = nc.dram_tensor("b1", [L, 128, NF], F32, kind="ExternalInput")
    if flags["bias_2"]:
        ext["b2b"] = nc.dram_tensor("b2b", [L, 128, D], F32, kind="ExternalInput")
    if flags["ln1"]:
        ext["ln1sb"] = nc.dram_tensor("ln1sb", [L, 2, 128, D], F32, kind="ExternalInput")
    if flags["ln2"]:
        ext["ln2sb"] = nc.dram_tensor("ln2sb", [L, 2, 128, D], F32, kind="ExternalInput")
    if flags["bl"]:
        ext["blb"] = nc.dram_tensor("blb", [128, VSH], F32, kind="ExternalInput")

    with tile.TileContext(nc) as tc:
        with ExitStack() as ctx:
            # ---- permanent pools ----
            perm = ctx.enter_context(tc.tile_pool(name="perm", bufs=1))
            hT_pool = ctx.enter_context(tc.tile_pool(name="hT", bufs=1))
            ln_pool = ctx.enter_context(tc.tile_pool(name="ln", bufs=4))

            h_tiles = [perm.tile([128, D], F32, tag=f"h{i}", name=f"h{i}") for i in range(NTO)]
            ident = perm.tile([128, 128], F32, tag="ident", name="ident")
            nc.gpsimd.memset(ident[:], 0.0)
            nc.gpsimd.affine_select(
                out=ident[:], in_=ident[:], compare_op=ALU.not_equal,
                fill=1.0, base=0, pattern=[[-1, 128]], channel_multiplier=1,
            )
            mask_sb = perm.tile([128, NTO, 2, 128], MM_DT, tag="mask", name="mask")
            nc.sync.dma_start(mask_sb[:], d_mask[:])
            ones_col = perm.tile([1, 128], MM_DT, tag="ones", name="ones")
            nc.vector.memset(ones_col[:], 1.0)

            czero = perm.tile([128, 1], F32, tag="czero", name="czero")
            ceps = perm.tile([128, 1], F32, tag="ceps", name="ceps")
            nc.vector.memset(czero[:], 0.0)
            nc.vector.memset(ceps[:], EPS)
            nc.const_aps.aps[(F32, 0.0)] = czero[:]
            nc.const_aps.aps[(F32, EPS)] = ceps[:]

            idx_sb = perm.tile([128, NTO], mybir.dt.int32, tag="idx", name="idx")
            np_sb = perm.tile([128, NTO], F32, tag="np", name="np")
            nc.sync.dma_start(idx_sb[:], d_idx[:])
            nc.sync.dma_start(np_sb[:], d_np[:])

            def layer_norm(sb_bcast):
                """In-place LN over h_tiles (own tiles)."""
                for i in range(NTO):
                    st = ln_pool.tile([128, 6], F32, tag="st", name="st")
                    mv = ln_pool.tile([128, 2], F32, tag="mv", name="mv")
                    nc.vector.bn_stats(out=st[:], in_=h_tiles[i][:])
                    nc.vector.bn_aggr(out=mv[:], in_=st[:])
                    t1 = ln_pool.tile([128, 1], F32, tag="t1", name="t1")
                    r = ln_pool.tile([128, 1], F32, tag="r", name="r")
                    nmr = ln_pool.tile([128, 1], F32, tag="nmr", name="nmr")
                    nc.scalar.activation(t1[:], mv[:, 1:2], AF.Ln, bias=EPS)
                    nc.scalar.activation(r[:], t1[:], AF.Exp, scale=-0.5)
                    nc.vector.tensor_scalar(
                        out=nmr[:], in0=mv[:, 0:1], scalar1=r[:, 0:1],
                        scalar2=-1.0, op0=ALU.mult, op1=ALU.mult,
                    )
                    nc.scalar.activation(
                        h_tiles[i][:], h_tiles[i][:], AF.Identity,
                        bias=nmr[:, 0:1], scale=r[:, 0:1],
                    )
                    if sb_bcast is not None:
                        s_t, b_t = sb_bcast
                        nc.vector.tensor_mul(h_tiles[i][:], h_tiles[i][:], s_t[:])
                        nc.vector.tensor_add(h_tiles[i][:], h_tiles[i][:], b_t[:])

            def transpose_h(ps_pool):
                """h (fp32 [t,d], own tiles) -> hT tiles [128, TO] MM_DT."""
                hT = [hT_pool.tile([128, TO], MM_DT, tag=f"hT{dc}", name=f"hT{dc}") for dc in range(ND)]
                for dc in range(ND):
                    for i in range(NTO):
                        pt = ps_pool.tile([128, 128], F32, tag="ps_x", name="ps_x")
                        nc.tensor.transpose(
                            pt[:], h_tiles[i][:, dc * 128 : (dc + 1) * 128], ident[:]
                        )
                        nc.vector.tensor_copy(
                            hT[dc][:, i * 128 : (i + 1) * 128], pt[:]
                        )
                return hT

            # ================= embedding =================
            with tc.tile_pool(name="emb", bufs=3) as emb_pool:
                for i in range(NTO):
                    g = emb_pool.tile([128, D], F32, tag="g", name="g")
                    nc.gpsimd.indirect_dma_start(
                        out=g[:], out_offset=None, in_=d_emb[:, :],
                        in_offset=bass.IndirectOffsetOnAxis(
                            ap=idx_sb[:, i : i + 1], axis=0
                        ),
                    )
                    p_t = emb_pool.tile([128, D], F32, tag="pe", name="pe")
                    nc.sync.dma_start(p_t[:], d_pe[i])
                    nc.vector.scalar_tensor_tensor(
                        out=h_tiles[i][:], in0=g[:],
                        scalar=np_sb[:, i : i + 1], in1=p_t[:],
                        op0=ALU.mult, op1=ALU.add,
                    )

            # ================= transformer body =================
            with ExitStack() as bctx:
                wqkvo_p = bctx.enter_context(tc.tile_pool(name="wqkvo", bufs=2))
                w12_p = bctx.enter_context(tc.tile_pool(name="w12", bufs=1))
                act_p = bctx.enter_context(tc.tile_pool(name="acts", bufs=1))
                attn_p = bctx.enter_context(tc.tile_pool(name="attn", bufs=6))
                sm_p = bctx.enter_context(tc.tile_pool(name="sm", bufs=4))
                bias_p = bctx.enter_context(tc.tile_pool(name="biasp", bufs=2))
                ps_mm = bctx.enter_context(
                    tc.tile_pool(name="ps_mm", bufs=3, space="PSUM")
                )
                ps_o = bctx.enter_context(
                    tc.tile_pool(name="ps_o", bufs=2, space="PSUM")
                )
                ps_x = bctx.enter_context(
                    tc.tile_pool(name="ps_x", bufs=2, space="PSUM")
                )

                for l in range(n_layers):
                    # ---- load layer weights ----
                    wq_sb = wqkvo_p.tile([128, ND, D], MM_DT, tag="wq", name="wq")
                    wk_sb = wqkvo_p.tile([128, ND, D], MM_DT, tag="wk", name="wk")
                    wv_sb = wqkvo_p.tile([128, ND, D], MM_DT, tag="wv", name="wv")
                    wo_sb = wqkvo_p.tile([128, ND, D], MM_DT, tag="wo", name="wo")
                    for w_sb, d_w in ((wq_sb, d_wq), (wk_sb, d_wk), (wv_sb, d_wv), (wo_sb, d_wo)):
                        nc.sync.dma_start(w_sb[:], d_w[l])
                    w1_sb = w12_p.tile([128, ND, F], MM_DT, tag="w1", name="w1")
                    w2_sb = w12_p.tile([128, NF, D], MM_DT, tag="w2", name="w2")
                    nc.sync.dma_start(w1_sb[:], d_w1[l])
                    nc.sync.dma_start(w2_sb[:], d_w2[l])

                    bias_aps = {}
                    if flags["bias_qkv"]:
                        bq_sb = bias_p.tile([128, ND], F32, tag="bq", name="bq")
                        bk_sb = bias_p.tile([128, ND], F32, tag="bk", name="bk")
                        bv_sb = bias_p.tile([128, D], F32, tag="bv", name="bv")
                        nc.sync.dma_start(bq_sb[:], ext["bq"][l])
                        nc.sync.dma_start(bk_sb[:], ext["bk"][l])
                        nc.sync.dma_start(bv_sb[:], ext["bvb"][l])
                        bias_aps.update(bq=bq_sb, bk=bk_sb, bv=bv_sb)
                    if flags["bias_o"]:
                        bo_sb = bias_p.tile([128, D], F32, tag="bo", name="bo")
                        nc.sync.dma_start(bo_sb[:], ext["bob"][l])
                        bias_aps["bo"] = bo_sb
                    if flags["bias_1"]:
                        b1_sb = bias_p.tile([128, NF], F32, tag="b1", name="b1")
                        nc.sync.dma_start(b1_sb[:], ext["b1"][l])
                        bias_aps["b1"] = b1_sb
                    if flags["bias_2"]:
                        b2_sb = bias_p.tile([128, D], F32, tag="b2", name="b2")
                        nc.sync.dma_start(b2_sb[:], ext["b2b"][l])
                        bias_aps["b2"] = b2_sb
                    ln_bcast = {}
                    for lnk in ("ln1", "ln2"):
                        if flags[lnk]:
                            s_t = bias_p.tile([128, D], F32, tag=f"{lnk}s", name=f"{lnk}s")
                            b_t = bias_p.tile([128, D], F32, tag=f"{lnk}b", name=f"{lnk}b")
                            nc.sync.dma_start(s_t[:], ext[f"{lnk}sb"][l, 0])
                            nc.sync.dma_start(b_t[:], ext[f"{lnk}sb"][l, 1])
                            ln_bcast[lnk] = (s_t, b_t)

                    # ---- hT (own tokens) ----
                    hT = transpose_h(ps_x)

                    # ---- kT then v (own tokens) -> stage to DRAM -> CC ----
                    kT_own = [act_p.tile([128, TO], MM_DT, tag=f"kO{m}", name=f"kO{m}") for m in range(ND)]
                    for m in range(ND):
                        ps = ps_mm.tile([128, TO], F32, tag="mm", name="mm")
                        for kc in range(ND):
                            nc.tensor.matmul(
                                ps[:],
                                lhsT=wk_sb[:, kc, m * 128 : (m + 1) * 128],
                                rhs=hT[kc][:],
                                start=(kc == 0), stop=(kc == ND - 1),
                            )
                        if "bk" in bias_aps:
                            nc.scalar.activation(
                                kT_own[m][:], ps[:], AF.Identity,
                                bias=bias_aps["bk"][:, m : m + 1],
                            )
                        else:
                            nc.scalar.copy(kT_own[m][:], ps[:])
                        nc.scalar.dma_start(
                            d_cc_in[l][:, m * TO : (m + 1) * TO], kT_own[m][:]
                        )

                    v_own = [act_p.tile([128, H, DK + 1], MM_DT, tag=f"vO{i}", name=f"vO{i}") for i in range(NTO)]
                    for i in range(NTO):
                        ps = ps_mm.tile([128, TO], F32, tag="mm", name="mm")
                        for kc in range(ND):
                            nc.tensor.matmul(
                                ps[:],
                                lhsT=hT[kc][:, i * 128 : (i + 1) * 128],
                                rhs=wv_sb[:, kc, :],
                                start=(kc == 0), stop=(kc == ND - 1),
                            )
                        src = ps[:].rearrange("p (h e) -> p h e", h=H)
                        nc.vector.tensor_copy(v_own[i][:, :, 0:DK], src)
                        if "bv" in bias_aps:
                            nc.vector.tensor_add(
                                v_own[i][:, :, 0:DK], v_own[i][:, :, 0:DK],
                                bias_aps["bv"][:].rearrange("p (h e) -> p h e", h=H),
                            )
                        nc.vector.memset(v_own[i][:, :, DK : DK + 1], 1.0)
                        nc.scalar.dma_start(
                            d_cc_in[l][:, ND * TO + i * VW : ND * TO + (i + 1) * VW],
                            v_own[i][:].rearrange("p h e -> p (h e)"),
                        )

                    nc.gpsimd.collective_compute(
                        "AllGather",
                        ALU.bypass,
                        replica_groups=GROUPS,
                        ins=[d_cc_in[l][:]],
                        outs=[d_cc_out[l][:]],
                    )

                    # ---- q (own tokens, overlaps the collective) ----
                    qT = [act_p.tile([128, TO], MM_DT, tag=f"qT{m}", name=f"qT{m}") for m in range(ND)]
                    for m in range(ND):
                        ps = ps_mm.tile([128, TO], F32, tag="mm", name="mm")
                        for kc in range(ND):
                            nc.tensor.matmul(
                                ps[:],
                                lhsT=wq_sb[:, kc, m * 128 : (m + 1) * 128],
                                rhs=hT[kc][:],
                                start=(kc == 0), stop=(kc == ND - 1),
                            )
                        if "bq" in bias_aps:
                            nc.scalar.activation(
                                qT[m][:], ps[:], AF.Identity,
                                bias=bias_aps["bq"][:, m : m + 1],
                            )
                        else:
                            nc.scalar.copy(qT[m][:], ps[:])

                    # ---- gather back in GLOBAL token order ----
                    kT_full = [act_p.tile([128, T], MM_DT, tag=f"kF{m}", name=f"kF{m}") for m in range(ND)]
                    v_full = [act_p.tile([128, H, DK + 1], MM_DT, tag=f"vF{g}", name=f"vF{g}") for g in range(NT)]
                    for p in range(2):
                        for m in range(ND):
                            # global tile 2j+p <- rank p local tile j
                            dst = kT_full[m][:].rearrange(
                                "q (a b c) -> q a b c", a=NTO, b=2
                            )[:, :, p, :]
                            src = d_cc_out[l][p, :, m * TO : (m + 1) * TO].rearrange(
                                "q (j c) -> q j c", j=NTO
                            )
                            nc.sync.dma_start(dst, src)
                        for j in range(NTO):
                            nc.sync.dma_start(
                                v_full[2 * j + p][:].rearrange("p h e -> p (h e)"),
                                d_cc_out[l][p, :, ND * TO + j * VW : ND * TO + (j + 1) * VW],
                            )

                    # ---- attention (own q tiles, global k tiles) ----
                    oT = [act_p.tile([128, TO], MM_DT, tag=f"oT{m}", name=f"oT{m}") for m in range(ND)]
                    for hd in range(H):
                        mk, r_off = hd // 2, (hd % 2) * DK
                        qT_h = qT[mk][r_off : r_off + DK, :]
                        pso = ps_o.tile([DK + 1, TO], F32, tag="po", name="po")
                        for kt in range(NT):
                            s = kt // 2          # first own-q tile attending kt
                            off = s * 128
                            at = attn_p.tile([128, TO], MM_DT, tag="at", name="at")
                            pss = ps_mm.tile([128, TO], F32, tag="mm", name="mm")
                            nc.tensor.matmul(
                                pss[:, off:TO],
                                lhsT=kT_full[mk][r_off : r_off + DK, kt * 128 : (kt + 1) * 128],
                                rhs=qT_h[:, off:TO],
                                start=True, stop=True,
                            )
                            nc.scalar.activation(
                                at[:, off:TO], pss[:, off:TO], AF.Exp,
                                scale=1.0 / np.sqrt(DK),
                            )
                            # mask the first suffix block (diag/future per rank)
                            nc.vector.tensor_mul(
                                at[:, off : off + 128],
                                at[:, off : off + 128],
                                mask_sb[:, s, kt % 2, :],
                            )
                            nc.tensor.matmul(
                                pso[:, off:TO],
                                lhsT=v_full[kt][:, hd, :],
                                rhs=at[:, off:TO],
                                start=(kt == 0), stop=(kt == NT - 1),
                            )
                        # normalize: oT = pso[0:64] * bcast(1/pso[64])
                        rr = sm_p.tile([1, TO], MM_DT, tag="rr", name="rr")
                        with nc.allow_low_precision(
                            reason="f32r denom row feeds PE broadcast"
                        ):
                            nc.vector.reciprocal(rr[:], pso[DK : DK + 1, :])
                        psb = ps_x.tile([128, TO], F32, tag="ps_x", name="ps_x")
                        nc.tensor.matmul(
                            psb[:],
                            lhsT=ones_col[:],
                            rhs=rr[:],
                            start=True, stop=True,
                        )
                        bc = sm_p.tile([128, TO], F32, tag="bc", name="bc")
                        nc.scalar.copy(bc[:], psb[:])
                        nc.vector.tensor_tensor(
                            out=oT[mk][r_off : r_off + DK, :],
                            in0=pso[0:DK, :], in1=bc[0:DK, :], op=ALU.mult,
                        )

                    # ---- Wo + residual ----
                    for i in range(NTO):
                        ps = ps_mm.tile([128, D], F32, tag="mm", name="mm")
                        for kc in range(ND):
                            nc.tensor.matmul(
                                ps[:],
                                lhsT=oT[kc][:, i * 128 : (i + 1) * 128],
                                rhs=wo_sb[:, kc, :],
                                start=(kc == 0), stop=(kc == ND - 1),
                            )
                        nc.vector.tensor_add(h_tiles[i][:], h_tiles[i][:], ps[:])
                        if "bo" in bias_aps:
                            nc.vector.tensor_add(
                                h_tiles[i][:], h_tiles[i][:], bias_aps["bo"][:]
                            )
                    layer_norm(ln_bcast.get("ln1"))

                    # ---- FFN ----
                    hT2 = transpose_h(ps_x)
                    ffT = [act_p.tile([128, TO], MM_DT, tag=f"ffT{m}", name=f"ffT{m}") for m in range(NF)]
                    for m in range(NF):
                        ps = ps_mm.tile([128, TO], F32, tag="mm", name="mm")
                        for kc in range(ND):
                            nc.tensor.matmul(
                                ps[:],
                                lhsT=w1_sb[:, kc, m * 128 : (m + 1) * 128],
                                rhs=hT2[kc][:],
                                start=(kc == 0), stop=(kc == ND - 1),
                            )
                        bias = (
                            bias_aps["b1"][:, m : m + 1]
                            if "b1" in bias_aps else 0.0
                        )
                        nc.scalar.activation(
                            ffT[m][:], ps[:], AF.Gelu, bias=bias,
                        )
                    for i in range(NTO):
                        ps = ps_mm.tile([128, D], F32, tag="mm", name="mm")
                        for kc in range(NF):
                            nc.tensor.matmul(
                                ps[:],
                                lhsT=ffT[kc][:, i * 128 : (i + 1) * 128],
                                rhs=w2_sb[:, kc, :],
                                start=(kc == 0), stop=(kc == NF - 1),
                            )
                        nc.vector.tensor_add(h_tiles[i][:], h_tiles[i][:], ps[:])
                        if "b2" in bias_aps:
                            nc.vector.tensor_add(
                                h_tiles[i][:], h_tiles[i][:], bias_aps["b2"][:]
                            )
                    layer_norm(ln_bcast.get("ln2"))

            # ================= final LN + LM head =================
            layer_norm(None)  # lnf scale/bias folded into Wl'/bl' on host
            with ExitStack() as lctx:
                ps_x2 = lctx.enter_context(
                    tc.tile_pool(name="ps_x2", bufs=2, space="PSUM")
                )
                # own final hT -> stage -> CC -> rebuild global hTf
                hTf_own = transpose_h(ps_x2)
                for m in range(ND):
                    nc.scalar.dma_start(
                        d_ccf_in[:, m * TO : (m + 1) * TO], hTf_own[m][:]
                    )
                nc.gpsimd.collective_compute(
                    "AllGather",
                    ALU.bypass,
                    replica_groups=GROUPS,
                    ins=[d_ccf_in[:]],
                    outs=[d_ccf_out[:]],
                )
                if not lm:
                    for i in range(NTO):
                        nc.sync.dma_start(
                            d_out[i * 128 : (i + 1) * 128, 0:D], h_tiles[i][:]
                        )
                else:
                    hTf_p = lctx.enter_context(tc.tile_pool(name="hTf", bufs=1))
                    hTf = [hTf_p.tile([128, T], MM_DT, tag=f"hTf{m}", name=f"hTf{m}") for m in range(ND)]
                    for p in range(2):
                        for m in range(ND):
                            dst = hTf[m][:].rearrange(
                                "q (a b c) -> q a b c", a=NTO, b=2
                            )[:, :, p, :]
                            src = d_ccf_out[p, :, m * TO : (m + 1) * TO].rearrange(
                                "q (j c) -> q j c", j=NTO
                            )
                            nc.sync.dma_start(dst, src)
                    wl_p = lctx.enter_context(tc.tile_pool(name="wl", bufs=2))
                    st_p = lctx.enter_context(tc.tile_pool(name="stage", bufs=3))
                    ps_lm = lctx.enter_context(
                        tc.tile_pool(name="ps_lm", bufs=6, space="PSUM")
                    )
                    blb_sb = None
                    if flags["bl"]:
                        blb_sb = wl_p.tile([128, VSH], F32, tag="blb", name="blb")
                        nc.sync.dma_start(blb_sb[:], ext["blb"][:])
                    for vg in range(VG):
                        wl_sb = wl_p.tile([128, ND, VGW], MM_DT, tag="wl", name="wl")
                        nc.sync.dma_start(wl_sb[:], d_wl[:, :, vg, :])
                        for tt in range(NT):
                            stage = st_p.tile([128, VGW], F32, tag="stage", name="stage")
                            for vc in range(NVC):
                                ps = ps_lm.tile([128, VC], F32, tag="lm", name="lm")
                                for kc in range(ND):
                                    nc.tensor.matmul(
                                        ps[:],
                                        lhsT=hTf[kc][:, tt * 128 : (tt + 1) * 128],
                                        rhs=wl_sb[:, kc, vc * VC : (vc + 1) * VC],
                                        start=(kc == 0), stop=(kc == ND - 1),
                                    )
                                dst = stage[:, vc * VC : (vc + 1) * VC]
                                if blb_sb is not None:
                                    nc.vector.tensor_add(
                                        dst, ps[:],
                                        blb_sb[:, vg * VGW + vc * VC : vg * VGW + (vc + 1) * VC],
                                    )
                                else:
                                    nc.vector.tensor_copy(dst, ps[:])
                            nc.scalar.dma_start(
                                d_out[tt * 128 : (tt + 1) * 128,
                                      vg * VGW : (vg + 1) * VGW],
                                stage[:],
                            )

    _split_waits(nc)
    return nc


# ----------------------------------------------------------------------------
# Host side
# ----------------------------------------------------------------------------

def _prep_weight(w):
    """[Din, N] -> [128, Din//128, N] in MM_DT (contraction chunks)."""
    Din, N = w.shape
    return np.ascontiguousarray(
        w.reshape(Din // 128, 128, N).transpose(1, 0, 2)
    ).astype(MM_NP)


def _host_inputs(inputs):
    x = np.asarray(inputs["x"]).astype(np.int64)
    embed = np.asarray(inputs["embed"], dtype=np.float32)
    lnf_s = np.asarray(inputs["lnf_s"], dtype=np.float64)
    lnf_b = np.asarray(inputs["lnf_b"], dtype=np.float64)
    Wl = np.asarray(inputs["Wl"], dtype=np.float64)
    bl = np.asarray(inputs["bl"], dtype=np.float64)

    # fold lnf scale/bias into the LM head (exact)
    Wl_f = (lnf_s[:, None] * Wl)
    bl_f = lnf_b @ Wl + bl

    flags = {
        "bias_qkv": not (
            np.all(inputs["bq"] == 0) and np.all(inputs["bk"] == 0)
            and np.all(inputs["bv"] == 0)
        ),
        "bias_o": not np.all(inputs["bo"] == 0),
        "bias_1": not np.all(inputs["b1"] == 0),
        "bias_2": not np.all(inputs["b2"] == 0),
        "ln1": not (np.all(inputs["ln1_s"] == 1) and np.all(inputs["ln1_b"] == 0)),
        "ln2": not (np.all(inputs["ln2_s"] == 1) and np.all(inputs["ln2_b"] == 0)),
        "bl": not np.all(bl_f == 0),
    }

    # positional encodings (input independent)
    pos = np.arange(T, dtype=np.float32)[:, None]
    div = np.exp(np.arange(0, D, 2, dtype=np.float32) * (-np.log(10000.0) / D))
    pe = np.zeros((T, D), np.float32)
    pe[:, 0::2] = np.sin(pos * div)
    pe[:, 1::2] = np.cos(pos * div)
    pe_r = np.ascontiguousarray(pe.reshape(NT, 128, D))

    wq = np.stack([_prep_weight(np.asarray(inputs["Wq"][l])) for l in range(L)])
    wk = np.stack([_prep_weight(np.asarray(inputs["Wk"][l])) for l in range(L)])
    wv = np.stack([_prep_weight(np.asarray(inputs["Wv"][l])) for l in range(L)])
    wo = np.stack([_prep_weight(np.asarray(inputs["Wo"][l])) for l in range(L)])
    w1 = np.stack([_prep_weight(np.asarray(inputs["W1"][l])) for l in range(L)])
    w2 = np.stack([_prep_weight(np.asarray(inputs["W2"][l])) for l in range(L)])

    # causal masks per rank: [128 k, NTO own-q-tile, parity, 128 q]
    tri = np.triu(np.ones((128, 128), np.float32))  # keep where q >= k
    m_rank = []
    for p in range(2):
        m = np.empty((128, NTO, 2, 128), np.float32)
        for i in range(NTO):
            if p == 0:
                m[:, i, 0] = tri          # kt == 2i: diagonal block
                m[:, i, 1] = 0.0          # kt == 2i+1: future (peer) block
            else:
                m[:, i, 0] = 1.0          # kt == 2i: full past block
                m[:, i, 1] = tri          # kt == 2i+1: diagonal block
        m_rank.append(m.astype(MM_NP))

    per_core = []
    for c in range(NCORES):
        b, vh = c % B, c // B
        wl_slice = Wl_f[:, vh * VSH : (vh + 1) * VSH].astype(np.float32)
        wl_r = np.ascontiguousarray(
            wl_slice.reshape(ND, 128, VG, VGW).transpose(1, 0, 2, 3)
        ).astype(MM_NP)
        xb = x[b].reshape(NT, 128)[vh::2]          # own token tiles [NTO,128]
        m = {
            "idx": np.ascontiguousarray(xb.T.astype(np.int32)),
            "notpad": np.ascontiguousarray(
                (xb != 0).astype(np.float32).T
            ),
            "pe": np.ascontiguousarray(pe_r[vh::2]),
            "embed": embed,
            "mask": m_rank[vh],
            "wq": wq, "wk": wk, "wv": wv, "wo": wo, "w1": w1, "w2": w2,
            "wl": wl_r,
        }
        if flags["bias_qkv"]:
            m["bq"] = np.stack([
                np.asarray(inputs["bq"][l]).reshape(ND, 128).T.astype(np.float32)
                for l in range(L)])
            m["bk"] = np.stack([
                np.asarray(inputs["bk"][l]).reshape(ND, 128).T.astype(np.float32)
                for l in range(L)])
            m["bvb"] = np.stack([
                np.broadcast_to(np.asarray(inputs["bv"][l], dtype=np.float32), (128, D)).copy()
                for l in range(L)])
        if flags["bias_o"]:
            m["bob"] = np.stack([
                np.broadcast_to(np.asarray(inputs["bo"][l], dtype=np.float32), (128, D)).copy()
                for l in range(L)])
        if flags["bias_1"]:
            m["b1"] = np.stack([
                np.asarray(inputs["b1"][l]).reshape(NF, 128).T.astype(np.float32)
                for l in range(L)])
        if flags["bias_2"]:
            m["b2b"] = np.stack([
                np.broadcast_to(np.asarray(inputs["b2"][l], dtype=np.float32), (128, D)).copy()
                for l in range(L)])
        for lnk, skey, bkey in (("ln1", "ln1_s", "ln1_b"), ("ln2", "ln2_s", "ln2_b")):
            if flags[lnk]:
                m[f"{lnk}sb"] = np.stack([
                    np.stack([
                        np.broadcast_to(np.asarray(inputs[skey][l], dtype=np.float32), (128, D)).copy(),
                        np.broadcast_to(np.asarray(inputs[bkey][l], dtype=np.float32), (128, D)).copy(),
                    ]) for l in range(L)])
        if flags["bl"]:
            m["blb"] = np.broadcast_to(
                bl_f[vh * VSH : (vh + 1) * VSH].astype(np.float32), (128, VSH)
            ).copy()
        per_core.append(m)
    return flags, per_core


_PROGRAM_CACHE = {}


def kernel(**inputs) -> np.ndarray:
    flags, per_core = _host_inputs(inputs)
    key = tuple(sorted(flags.items()))
    if key not in _PROGRAM_CACHE:
        _PROGRAM_CACHE[key] = build_program(flags)
    nc = _PROGRAM_CACHE[key]
    res = run_bass_kernel_spmd(nc, per_core, list(range(NCORES)))
    out = np.empty((B, T, V), np.float32)
    for c in range(NCORES):
        b, vh = c % B, c // B
        out[b, :, vh * VSH : (vh + 1) * VSH] = res.results[c]["logits"]
    return out
